# revision 4
# baseline (speedup 1.0000x reference)
"""Trainium2 Bass kernel for ContextQueryAttention (BiDAF-style trilinear attention).

Computes, per batch n:
    sim[c,q] = <ctx[c], wc> + <xq[q], wc> + <ctx[c] * wcq, xq[q]>
    c2q  = softmax_q(sim) @ xq                      # [C, F]
    q2c  = softmax_c(max_q sim) @ ctx               # [F]
    out  = concat([ctx, c2q, ctx*c2q, ctx*q2c], -1) # [C, 4F]

Sharding: data-parallel over batch N=64 across 8 NeuronCores (8 batches/core).

Per-core structure (per batch):
  - all PE matmuls in fp16 (fp32 runs at 4 cyc/row vs fp16 1 cyc/row; fp16's
    10 mantissa bits keep logit noise ~0.02 abs on std-32 rows so softmax
    weights are stable; l2 err ~1e-3, far under the 2e-2 gate); PSUM fp32
  - ctx cast to fp16 (ctxh) for PE use; fp32 ctx kept for term1/3/4 + stores
  - ctxT built via 32 fp16 PE transposes (f-contraction needs f-major operands)
  - sim psum [128c, 129] per c-tile: 4 K-chunk matmuls with an augmented
    moving operand [wcq*xqT | wc] so column 128 accumulates s_ctx for free,
    plus a rank-1 (ones x s_qry) matmul for the query bias term
  - softmax over q on the free axis: DVE reduce_max(negate) -> ACT exp with
    per-partition bias, fp16 E out, fp32 accumulated row-sum
  - q2c chain (gpsimd cross-partition max) issued right after pass 1; its PE
    matmuls issued after 2 c2q tiles so the PE streams during the all-reduce
  - pass 2 per tile: E^T -> c2q matmul -> normalize / term3 / term4 -> ONE
    [128, 1536] store per tile, so output DMA flows through the whole batch;
    the NEXT batch's ctx casts + ctxT transposes are interleaved between
    pass-2 tiles so pass 1 of b+1 starts with everything ready
  - ctx (term1) stored as one merged DMA per batch during pass 1
  - loads ride the ACT HWDGE ring; stores the SP ring
"""

import os

os.environ.setdefault("JAX_PLATFORMS", "axon")

import numpy as np

import concourse.bass as bass
import concourse.mybir as mybir
import concourse.tile as tile
from concourse import bacc, bass_isa, bass_utils
from concourse.masks import make_identity

f32 = mybir.dt.float32
f16 = mybir.dt.float16
AX = mybir.AxisListType.X
EXP = mybir.ActivationFunctionType.Exp
COPY = mybir.ActivationFunctionType.Copy
MULT = mybir.AluOpType.mult
ADD = mybir.AluOpType.add

N_CORES = 8
B = 8          # batches per core
C = 1024       # context length
Q = 128        # query length
F = 512        # feature dim
CT = C // 128  # c-tiles per batch
FC = F // 128  # f-chunks


def build_nc():
    nc = bacc.Bacc("TRN2", target_bir_lowering=False, debug=False)
    xc = nc.dram_tensor("x_context", [B, C, F], f32, kind="ExternalInput").ap()
    xq_d = nc.dram_tensor("x_query", [B, Q, F], f32, kind="ExternalInput").ap()
    wc_d = nc.dram_tensor("w_context", [F], f32, kind="ExternalInput").ap()
    wcq_d = nc.dram_tensor("w_cq", [F], f32, kind="ExternalInput").ap()
    out = nc.dram_tensor("out", [B, C, 4 * F], f32, kind="ExternalOutput").ap()

    from contextlib import ExitStack

    with tile.TileContext(nc) as tc, ExitStack() as es:
        def pool(name, bufs, space="SBUF"):
            return es.enter_context(tc.tile_pool(name=name, bufs=bufs, space=space))

        const = pool("const", 1)
        ctx_p = pool("ctx_p", 3)
        ctxh_p = pool("ctxh_p", 2)
        ctxT_p = pool("ctxT_p", 2)
        xq_p = pool("xq_p", 3)
        xqh_p = pool("xqh_p", 2)
        xqw_p = pool("xqw_p", 2)
        tmp_p = pool("tmp_p", 2)
        e_p = pool("e_p", CT + 2)
        et_p = pool("et_p", 3)
        asm_p = pool("asm_p", 4)
        vec_p = pool("vec_p", CT + 2)
        sml_p = pool("sml_p", 2)
        ps_sim_p = pool("ps_sim", 2, "PSUM")
        ps_ctxT_p = pool("ps_ctxT", 2, "PSUM")
        ps_c2q_p = pool("ps_c2q", 2, "PSUM")
        ps_sml_p = pool("ps_sml", 2, "PSUM")

        # loads on the ACT HWDGE ring; stores on the SP ring
        dma_load = nc.scalar.dma_start
        dma_store = nc.sync.dma_start

        ident = const.tile([128, 128], f32)
        make_identity(nc, ident)
        identh = const.tile([128, 128], f16)
        nc.vector.tensor_copy(identh, ident)
        ones_rowh = const.tile([1, 128], f16)
        nc.vector.memset(ones_rowh, 1.0)
        ones_col = const.tile([128, 1], f32)
        nc.vector.memset(ones_col, 1.0)
        wc_sb = const.tile([128, FC], f32)
        dma_load(wc_sb, wc_d.rearrange("(a p) -> p a", p=128))
        wc_sbh = const.tile([128, FC], f16)
        nc.vector.tensor_copy(wc_sbh, wc_sb)
        wcq_sb = const.tile([128, FC], f32)
        dma_load(wcq_sb, wcq_d.rearrange("(a p) -> p a", p=128))
        wc_row = const.tile([1, F], f32)
        dma_load(wc_row, wc_d[None, :])
        wc_rowh = const.tile([1, F], f16)
        nc.vector.tensor_copy(wc_rowh, wc_row)
        # wc broadcast along partitions (for s_qry): ones[1,128]^T @ wc[1,512]
        ps_wcb = ps_sml_p.tile([128, F], f32, tag="sml")
        nc.tensor.matmul(ps_wcb, lhsT=ones_rowh, rhs=wc_rowh, start=True, stop=True)
        wc_bc = const.tile([128, F], f32)
        nc.vector.tensor_copy(wc_bc, ps_wcb)

        def load_batch(b):
            ctx = ctx_p.tile([128, CT, F], f32, name="ctx")
            dma_load(ctx, xc[b].rearrange("(t p) f -> p t f", p=128))
            xq = xq_p.tile([128, F], f32, name="xq")
            dma_load(xq, xq_d[b])
            return ctx, xq

        # fp32 -> fp16 cast helpers, spread over engines
        def cast_dve(dst, src):
            nc.vector.tensor_scalar_mul(dst, src, 1.0)

        def cast_act(dst, src):
            nc.scalar.copy(dst, src)

        # ---- per-batch stage builders ----

        def xq_prep(xq):
            """xqh, xqw_aug (scaled+augmented xqT), s_qry row."""
            xqh = xqh_p.tile([128, F], f16, name="xqh")
            cast_dve(xqh, xq)
            xqw_aug = xqw_p.tile([128, FC, Q + 1], f16)
            for fc in range(FC):
                ps_xqT = ps_sml_p.tile([128, 128], f16, tag="sml")
                nc.tensor.transpose(ps_xqT, xqh[:, fc * 128 : (fc + 1) * 128], identh)
                nc.scalar.activation(
                    xqw_aug[:, fc, 0:Q], ps_xqT, COPY,
                    scale=wcq_sb[:, fc : fc + 1],
                )
                nc.vector.tensor_copy(
                    xqw_aug[:, fc, Q : Q + 1], wc_sbh[:, fc : fc + 1]
                )
            scr = tmp_p.tile([128, F], f32, name="scr", tag="scr")
            sq_col = vec_p.tile([128, 1], f32, tag="sqcol")
            nc.vector.tensor_mul(scr, xq, wc_bc)
            nc.vector.reduce_sum(sq_col, scr, axis=AX)
            ps_sqT = ps_sml_p.tile([1, 128], f32, tag="sml")
            nc.tensor.transpose(ps_sqT, sq_col, ident)
            sq_rowh = sml_p.tile([1, 128], f16, name="sq_rowh", tag="sq_row")
            nc.scalar.copy(sq_rowh, ps_sqT)
            return xqh, xqw_aug, sq_rowh

        def ctxT_chunk(ctx, ctxh, ctxT, half):
            """Cast 4 c-tiles to fp16 and transpose them into ctxT."""
            for j in range(4):
                t = half * 4 + j
                (cast_act if j % 2 == 0 else cast_dve)(ctxh[:, t], ctx[:, t])
            for fc in range(FC):
                ps_ct = ps_ctxT_p.tile([128, 512], f16)
                for j in range(4):
                    t = half * 4 + j
                    nc.tensor.transpose(
                        ps_ct[:, j * 128 : (j + 1) * 128],
                        ctxh[:, t, fc * 128 : (fc + 1) * 128],
                        identh,
                    )
                cp = nc.vector.tensor_copy if fc % 2 == 0 else nc.scalar.copy
                cp(ctxT[:, fc, half * 512 : (half + 1) * 512], ps_ct)

        def pass1_tile(t, ctxT, xqw_aug, sq_rowh, z, Es, rcps):
            ps_sim = ps_sim_p.tile([128, Q + 1], f32)
            for fc in range(FC):
                nc.tensor.matmul(
                    ps_sim,
                    lhsT=ctxT[:, fc, t * 128 : t * 128 + 128],
                    rhs=xqw_aug[:, fc],
                    start=(fc == 0),
                    stop=False,
                )
            nc.tensor.matmul(
                ps_sim[:, 0:Q], lhsT=ones_rowh, rhs=sq_rowh, start=False, stop=True
            )
            nmax = vec_p.tile([128, 1], f32, tag="nmax")
            nc.vector.reduce_max(nmax, ps_sim[:, 0:Q], axis=AX, negate=True)
            E = e_p.tile([128, Q], f16)
            rsum = vec_p.tile([128, 1], f32, tag="rsum")
            nc.scalar.activation(E, ps_sim[:, 0:Q], EXP, bias=nmax, accum_out=rsum)
            rcp = vec_p.tile([128, 1], f32, tag="rcp")
            nc.vector.reciprocal(rcp, rsum)
            # z[:, t] = s_ctx + rowmax = psum[:,128] - (-max)
            nc.vector.tensor_sub(z[:, t : t + 1], ps_sim[:, Q : Q + 1], nmax)
            Es.append(E)
            rcps.append(rcp)

        def q2c_prep(z):
            zmax = vec_p.tile([128, 1], f32, tag="zmax")
            nc.vector.reduce_max(zmax, z, axis=AX)
            gmax = vec_p.tile([128, 1], f32, tag="gmax")
            nc.gpsimd.partition_all_reduce(
                gmax, zmax, channels=128, reduce_op=bass_isa.ReduceOp.max
            )
            negb = vec_p.tile([128, 1], f32, tag="negb")
            nc.vector.tensor_scalar_mul(negb, gmax, -1.0)
            expz = sml_p.tile([128, CT], f16, name="expz", tag="expz")
            ers = vec_p.tile([128, 1], f32, tag="ers")
            nc.scalar.activation(expz, z, EXP, bias=negb, accum_out=ers)
            return expz, ers

        def q2c_matmuls(expz, ers, ctxh):
            ps_S = ps_sml_p.tile([1, 1], f32, tag="sml")
            nc.tensor.matmul(ps_S, lhsT=ers, rhs=ones_col, start=True, stop=True)
            rS = sml_p.tile([1, 1], f32, name="rS", tag="rS")
            nc.vector.reciprocal(rS, ps_S)
            ps_q2c = ps_sml_p.tile([1, F], f32, tag="sml")
            for t in range(CT):
                nc.tensor.matmul(
                    ps_q2c,
                    lhsT=expz[:, t : t + 1],
                    rhs=ctxh[:, t],
                    start=(t == 0),
                    stop=(t == CT - 1),
                )
            xq2c = sml_p.tile([1, F], f16, name="xq2c", tag="xq2c")
            nc.scalar.activation(xq2c, ps_q2c, COPY, scale=rS)
            ps_bc = ps_sml_p.tile([128, F], f32, tag="sml")
            nc.tensor.matmul(ps_bc, lhsT=ones_rowh, rhs=xq2c, start=True, stop=True)
            xq2cb = tmp_p.tile([128, F], f32, name="xq2cb", tag="xq2cb")
            nc.vector.tensor_copy(xq2cb, ps_bc)
            return xq2cb

        def stage_et(t, Es):
            ps_et = ps_sml_p.tile([128, Q], f16, tag="sml")
            nc.tensor.transpose(ps_et, Es[t], identh)
            ET = et_p.tile([128, Q], f16)
            nc.scalar.copy(ET, ps_et)
            return ET

        def stage_c2q(t, ET, ctx, xqh, rcps):
            ps_c2q = ps_c2q_p.tile([128, F], f32)
            nc.tensor.matmul(ps_c2q, lhsT=ET, rhs=xqh, start=True, stop=True)
            asm = asm_p.tile([128, 3 * F], f32)
            # normalized c2q, fused into the psum->sbuf move
            if t % 2 == 0:
                nc.scalar.activation(asm[:, 0:F], ps_c2q, COPY, scale=rcps[t])
            else:
                nc.vector.tensor_scalar_mul(asm[:, 0:F], ps_c2q, rcps[t])
            eng3 = nc.vector if t % 4 != 3 else nc.gpsimd
            eng3.tensor_mul(asm[:, F : 2 * F], ctx[:, t], asm[:, 0:F])
            return asm

        def stage_term4_store(b, t, asm, ctx, xq2cb):
            eng = nc.vector if t % 2 == 0 else nc.gpsimd
            eng.tensor_mul(asm[:, 2 * F : 3 * F], ctx[:, t], xq2cb)
            dma_store(out[b, t * 128 : (t + 1) * 128, F : 4 * F], asm)

        # ---- main software-pipelined loop ----
        nxt = load_batch(0)
        ctxh_nxt = ctxh_p.tile([128, CT, F], f16, name="ctxh")
        ctxT_nxt = ctxT_p.tile([128, FC, C], f16)
        ctxT_chunk(nxt[0], ctxh_nxt, ctxT_nxt, 0)
        ctxT_chunk(nxt[0], ctxh_nxt, ctxT_nxt, 1)

        for b in range(B):
            ctx, xq = nxt
            ctxh, ctxT = ctxh_nxt, ctxT_nxt
            if b + 1 < B:
                nxt = load_batch(b + 1)

            # term1: merged ctx store, flows during pass 1
            dma_store(out[b, :, 0:F].rearrange("(t p) f -> p t f", p=128), ctx)

            xqh, xqw_aug, sq_rowh = xq_prep(xq)

            # pass 1
            z = sml_p.tile([128, CT], f32, name="z", tag="z")
            Es = []
            rcps = []
            for t in range(CT):
                pass1_tile(t, ctxT, xqw_aug, sq_rowh, z, Es, rcps)

            expz, ers = q2c_prep(z)

            if b + 1 < B:
                ctxh_nxt = ctxh_p.tile([128, CT, F], f16, name="ctxh")
                ctxT_nxt = ctxT_p.tile([128, FC, C], f16)

            # pass 2, with next batch's ctxT build interleaved
            ET0 = stage_et(0, Es)
            ET1 = stage_et(1, Es)
            asm0 = stage_c2q(0, ET0, ctx, xqh, rcps)
            asm1 = stage_c2q(1, ET1, ctx, xqh, rcps)
            xq2cb = q2c_matmuls(expz, ers, ctxh)
            stage_term4_store(b, 0, asm0, ctx, xq2cb)
            if b + 1 < B:
                ctxT_chunk(nxt[0], ctxh_nxt, ctxT_nxt, 0)
            stage_term4_store(b, 1, asm1, ctx, xq2cb)
            prev = None
            for t in range(2, CT):
                ET = stage_et(t, Es)
                if prev is not None:
                    asm = stage_c2q(prev[0], prev[1], ctx, xqh, rcps)
                    stage_term4_store(b, prev[0], asm, ctx, xq2cb)
                if t == 4 and b + 1 < B:
                    ctxT_chunk(nxt[0], ctxh_nxt, ctxT_nxt, 1)
                prev = (t, ET)
            asm = stage_c2q(prev[0], prev[1], ctx, xqh, rcps)
            stage_term4_store(b, prev[0], asm, ctx, xq2cb)

    nc.compile()
    return nc


_NC = None


def kernel(**inputs):
    global _NC
    if _NC is None:
        _NC = build_nc()
    xc = np.ascontiguousarray(np.asarray(inputs["x_context"], dtype=np.float32))
    xq = np.ascontiguousarray(np.asarray(inputs["x_query"], dtype=np.float32))
    wc = np.ascontiguousarray(np.asarray(inputs["w_context"], dtype=np.float32))
    wcq = np.ascontiguousarray(np.asarray(inputs["w_cq"], dtype=np.float32))
    in_maps = [
        {
            "x_context": xc[i * B : (i + 1) * B],
            "x_query": xq[i * B : (i + 1) * B],
            "w_context": wc,
            "w_cq": wcq,
        }
        for i in range(N_CORES)
    ]
    res = bass_utils.run_bass_kernel_spmd(_NC, in_maps, core_ids=list(range(N_CORES)))
    return np.concatenate([res.results[i]["out"] for i in range(N_CORES)], axis=0)
